# revision 1
# baseline (speedup 1.0000x reference)
"""Trainium2 Bass kernel for nn_Encoder_21964462752332.

Math: the swap-test quantum circuit per 4x4 patch p (16 values) reduces to
    out = 0.5 + 0.5 * ||A p||^2 / ||p||^2,
where U (16x16, orthogonal) is the MPS block-circuit matrix built from the 12
weights_mps floats and A = U[:4, :].  Proof sketch: the MPS layers act only on
the 4 data wires (-> v = U p_hat), the CSWAP pair + Hadamards implement a swap
test of data wires (0,1) against the |00> discarded wires, giving
P(0) = (1 + sum_{j<4} v_j^2) / 2.  Orthogonality of U gives ||p||^2 = ||U p||^2,
so one matmul y = U p yields both numerator (first 4 rows) and denominator
(all 16 rows):  out = (num + den) / (2 den) with num = sum_{j<4} y_j^2,
den = sum_j y_j^2.

Device kernel (SPMD over 8 cores, patches sharded; f32r matmuls):
  x[128, F] : 8 patch-octets x 16 taps in partitions, F patches/octet in free
  y = blockdiag(U^T)^T x           (TensorE, 128x128, float32r)
  ysq = y^2                        (ScalarE activation Square, PSUM->SBUF)
  z = W2^T ysq                     (TensorE; z[o]=num+den, z[32+o]=2*den)
  out = z[0:8] * recip(z[32:40])   (ACT copy out of PSUM, then DVE)
Input ships as one host-packed "blob" (w1|w2|x) in a chunk-major contiguous
HBM layout (strided partition reads measured ~99 GB/s/core; contiguous ~4x).
Measured on trn2: rel err 1.38e-4 vs reference, ~18.1 us/core steady state.
"""

import numpy as np

# ---- problem geometry (hardcoded per contract) ----
BS = 256
H = W = 64
K = 4
S = 2
OH = OW = 31
N_PATCH = BS * OH * OW          # 246016
N_CORES = 8
NPC = N_PATCH // N_CORES        # 30752 patches per core
OCTETS = 8
F = NPC // OCTETS               # 3844 free columns per octet
CHUNKS = [512] * 7 + [F - 512 * 7]   # 7x512 + 260 (PSUM bank = 512 f32)

_CACHE = {}
TRACE = False            # test.py sets this to profile
TRACE_KWARGS = {}


def _build_U(weights_mps: np.ndarray) -> np.ndarray:
    """16x16 orthogonal MPS circuit matrix; amp index bits are MSB-first in
    local data-wire order (wire 0 = most significant)."""
    Wm = np.asarray(weights_mps, dtype=np.float64)
    I2 = np.eye(2)
    CNOT = np.array(
        [[1, 0, 0, 0], [0, 1, 0, 0], [0, 0, 0, 1], [0, 0, 1, 0]], dtype=np.float64
    )

    def ry(t):
        c, s = np.cos(t / 2.0), np.sin(t / 2.0)
        return np.array([[c, -s], [s, c]])

    def emb1(U2, w):
        out = np.array([[1.0]])
        for i in range(4):
            out = np.kron(out, U2 if i == w else I2)
        return out

    def emb2(U4, w):
        return np.kron(np.eye(2 ** w), np.kron(U4, np.eye(2 ** (2 - w))))

    U = np.eye(16)
    for l in range(2):
        for b in range(3):
            U = emb1(ry(Wm[l, b, 0]), b) @ U
            U = emb1(ry(Wm[l, b, 1]), b + 1) @ U
            U = emb2(CNOT, b) @ U
    return U


W_COLS = 168  # blob layout: [0:128)=w1, [128:168)=w2 (M=40), [168:168+F)=x
NBLK = (W_COLS + F + 511) // 512   # 4012 cols -> 8 contiguous 512-wide blocks


MM_DTYPE = "f32r"        # "fp32" (4 cyc/row) or "f32r" (1 cyc/row at N>=256)
FINAL = "act_copies"     # "act_copies" or "dve_direct"


def _build_bass(loop_reps=None, loop_unroll=1, mm_dtype=None, final=None,
                empty=False):
    import concourse.bacc as bacc
    import concourse.mybir as mybir
    from concourse.tile import TileContext

    mm_dtype = mm_dtype or MM_DTYPE
    final = final or FINAL
    f32 = mybir.dt.float32
    mmdt = {"fp32": f32, "f32r": mybir.dt.float32r}[mm_dtype]
    # Bacc (not plain Bass): its compile() runs generate_event_semaphores,
    # which splits multi-sem waits into EventSemaphore instructions --
    # walrus here only accepts 1 sync wait per engine instruction.
    nc = bacc.Bacc(None)
    # For f32r the whole input chain is declared float32r (bytes identical
    # to fp32; PE rounds mantissas) so the BIR verifier sees rounded
    # producers feeding FP32r matmuls.
    # chunk-major contiguous HBM layout: block i holds SBUF cols
    # [i*512,(i+1)*512) as a contiguous [128,512] slab (strided partition
    # reads measured ~99 GB/s/core; contiguous blocks are >4x faster)
    blob = nc.dram_tensor("blob", [NBLK, 128, 512], mmdt, kind="ExternalInput")
    out = nc.dram_tensor("out", [8, F], f32, kind="ExternalOutput")

    with TileContext(nc) as tc:
        with (
            tc.tile_pool(name="big", bufs=1) as bigpool,
            # one SBUF slot per chunk: no tile reuse -> no WAR deps -> every
            # instruction needs at most one sync wait (walrus limit here)
            tc.tile_pool(name="work", bufs=len(CHUNKS)) as wpool,
            tc.tile_pool(name="psum", bufs=4, space="PSUM") as ppool,
        ):
            # whole input staged in SBUF; per-chunk DMAs each carry one sem so
            # the consuming matmul never needs >1 sync wait (walrus limit).
            blobt = bigpool.tile([128, NBLK * 512], mmdt)
            w1v = blobt[:, 0:128]
            w2v = blobt[:, 128:168]

            def load():
                # contiguous block loads; compute overlaps the transfer
                for i in range(NBLK):
                    nc.sync.dma_start(
                        out=blobt[:, i * 512:(i + 1) * 512], in_=blob[i]
                    )

            def body():
                res = bigpool.tile([8, F], f32, tag="res")
                c0 = 0
                for cw in CHUNKS:
                    xv = blobt[:, W_COLS + c0:W_COLS + c0 + cw]

                    yp = ppool.tile([128, cw], f32, tag="y")
                    nc.tensor.matmul(
                        yp[:], lhsT=w1v, rhs=xv, start=True, stop=True,
                    )

                    ysq = wpool.tile([128, cw], mmdt, tag="ysq")
                    nc.scalar.activation(
                        ysq[:], yp[:], mybir.ActivationFunctionType.Square
                    )

                    zp = ppool.tile([40, cw], f32, tag="z")
                    nc.tensor.matmul(
                        zp[:], lhsT=w2v, rhs=ysq[:], start=True, stop=True,
                    )

                    if final == "act_copies":
                        # one ACT copy moves num (parts 0..7) and den (parts
                        # 32..39) out of PSUM; DVE then reads SBUF only
                        # (DVE reading PSUM crashes this HW/toolchain).
                        zs = wpool.tile([40, cw], f32, tag="zs")
                        nc.scalar.copy(zs[:], zp[:])
                        rden = wpool.tile([8, cw], f32, tag="rden")
                        nc.vector.reciprocal(rden[:], zs[32:40, :])
                        nc.vector.tensor_tensor(
                            res[:, c0:c0 + cw], zs[0:8, :], rden[:],
                            mybir.AluOpType.mult,
                        )
                    else:  # dve_direct: read PSUM directly, no ACT copies
                        rden = wpool.tile([8, cw], f32, tag="rden")
                        nc.vector.reciprocal(rden[:], zp[32:40, :])
                        nc.vector.tensor_tensor(
                            res[:, c0:c0 + cw], zp[0:8, :], rden[:],
                            mybir.AluOpType.mult,
                        )
                    c0 += cw
                nc.gpsimd.dma_start(out=out[:], in_=res[:])

            if loop_reps is None:
                load()
                body()
            else:
                with tc.For_i(0, loop_reps, 1):
                    for _ in range(loop_unroll):
                        load()
                        body()
    nc.compile()
    return nc


def _get_bass():
    if "nc" not in _CACHE:
        _CACHE["nc"] = _build_bass()
    return _CACHE["nc"]


def _prep_inputs(img, weights_mps):
    img = np.ascontiguousarray(np.asarray(img, dtype=np.float32))
    U = _build_U(weights_mps)

    # host-side weight prep (12 floats -> 16x16): w1 = blockdiag(U^T) x 8
    w1 = np.zeros((128, 128), dtype=np.float32)
    Ut = U.T.astype(np.float32)
    for o in range(OCTETS):
        w1[o * 16:(o + 1) * 16, o * 16:(o + 1) * 16] = Ut
    # mm2 weight, M=40: cols 0..7 -> num+den per octet, cols 32..39 -> 2*den
    # (den block at output partition 32 so the DVE divide's second operand
    # starts at a legal partition base)
    w2 = np.zeros((128, 40), dtype=np.float32)
    for o in range(OCTETS):
        w2[o * 16:o * 16 + 4, o] = 2.0
        w2[o * 16 + 4:(o + 1) * 16, o] = 1.0
        w2[o * 16:(o + 1) * 16, 32 + o] = 2.0

    # host-side im2col + pack: X[core, 128=o*16+tap, F]
    I = img[:, 0]
    pat = np.empty((BS, OH, OW, 16), dtype=np.float32)
    for kh in range(K):
        for kw in range(K):
            pat[..., kh * K + kw] = I[:, kh:kh + S * OH:S, kw:kw + S * OW:S]
    X = (
        pat.reshape(N_CORES, OCTETS, F, 16)
        .transpose(0, 1, 3, 2)
        .reshape(N_CORES, 128, F)
    )
    blobs = np.concatenate(
        [np.broadcast_to(np.concatenate([w1, w2], axis=1), (N_CORES, 128, W_COLS)), X],
        axis=2,
    )
    # pad to NBLK*512 cols and reorder chunk-major: [core, NBLK, 128, 512]
    pad = NBLK * 512 - (W_COLS + F)
    blobs = np.concatenate(
        [blobs, np.zeros((N_CORES, 128, pad), np.float32)], axis=2
    )
    blobs = blobs.reshape(N_CORES, 128, NBLK, 512).transpose(0, 2, 1, 3)
    return np.ascontiguousarray(blobs)


def kernel(img: np.ndarray, weights_mps: np.ndarray) -> np.ndarray:
    from concourse.bass_utils import run_bass_kernel_spmd

    blobs = _prep_inputs(img, weights_mps)
    nc = _get_bass()
    in_maps = [{"blob": blobs[c]} for c in range(N_CORES)]
    r = run_bass_kernel_spmd(
        nc, in_maps, list(range(N_CORES)), trace=TRACE, **TRACE_KWARGS
    )
    if TRACE:
        _CACHE["last_result"] = r

    outs = np.stack([r.results[c]["out"] for c in range(N_CORES)])  # (8, 8, F)
    return outs.reshape(N_PATCH).reshape(BS, 1, OH * OW).astype(np.float32)



# revision 20
# speedup vs baseline: 1.8095x; 1.8095x over previous
"""Trainium2 Bass kernel for nn_Encoder_21964462752332 (parity-plane rewrite).

Math: the swap-test circuit per 4x4 patch p reduces to
    out = 0.5 + 0.5 * ||A p||^2 / ||p||^2 = (num + den) / (2 den),
with A = U[:4, :], num = ||A p||^2, den = ||p||^2 (U = 16x16 MPS orthogonal
matrix built from the 12 weights_mps floats; see _build_U).

Dataflow (per core, 32 images, SPMD over 8 cores):
  The stride-2 / kernel-4 patch extraction is re-expressed over the four
  image parity planes Pl[pi,pj][r,c] = img[2r+pi, 2c+pj] (32x32 each).
  Patch (oh,ow) tap (kh,kw) = Pl[kh%2,kw%2][oh+kh//2, ow+kw//2], so with a
  [128, 1056] planes tile (partition = image*4 + plane, col = 32*r + c,
  pixel grid padded to 32 cols incl. a garbage ow=31 so every shifted view
  is a CONTIGUOUS column range):
    q          : 4 shift-matmuls, blockdiag(A-slice) weights  -> PSUM
    q^2        : ACT Square PSUM->SBUF (bf16)
    planes^2   : DVE tensor_tensor (bf16, 2x mode)
    num+den,
    2*den      : one PSUM accumulation group = 4 shift-ones-matmuls over
                 planes^2 (M=64: rows 0:32 get 1x, rows 32:64 get 2x)
                 + 1 ones-matmul over q^2 (rows 0:32)
    out        : ACT copy PSUM->SBUF, DVE reciprocal + multiply -> bf16
  Raw pixels ship once in bf16 (~0.3 MB/core vs 2.1 MB im2col f32 before);
  all matmuls bf16 (1 cyc/row).  Output ships bf16, host upcasts.
  A few zero-weight warm-up matmuls at t=0 ramp the PE p-state early.
"""

import numpy as np
import ml_dtypes

# ---- problem geometry (hardcoded per contract) ----
BS = 256
H = W = 64
OH = OW = 31
N_CORES = 8
NI = BS // N_CORES              # 32 images per core
GRID = 32 * 31                  # padded pixel grid cols (ow=31 is garbage)
PCOL0 = 576                     # planes tile offset inside P (after weights)
PCOLS = 1056                    # 1024 real plane cols + 32 pad
SHIFTS = [(0, 0), (0, 1), (1, 0), (1, 1)]
# output-row chunks (r0, nrows): N = nrows*32 <= 512 (PSUM bank); the last
# chunk is small so the post-matmul ACT/DVE/DMA tail is short
CHUNKS = [(0, 16), (16, 15)]
QW = GRID // 4                  # mm2 column-strip (quarter) width = 248
D0 = PCOL0 + 576                # first DMA: weights + planes rows 0..18

_CACHE = {}
TRACE = False            # test.py sets this to profile
TRACE_KWARGS = {}

WARM_MMS = 13            # PE ramp warm-up matmuls on zeroed SBUF
WARM_N = 256
OUT_DMAS = [(0, 512), (512, GRID)]


def _build_U(weights_mps: np.ndarray) -> np.ndarray:
    """16x16 orthogonal MPS circuit matrix; amp index bits are MSB-first in
    local data-wire order (wire 0 = most significant)."""
    Wm = np.asarray(weights_mps, dtype=np.float64)
    I2 = np.eye(2)
    CNOT = np.array(
        [[1, 0, 0, 0], [0, 1, 0, 0], [0, 0, 0, 1], [0, 0, 1, 0]], dtype=np.float64
    )

    def ry(t):
        c, s = np.cos(t / 2.0), np.sin(t / 2.0)
        return np.array([[c, -s], [s, c]])

    def emb1(U2, w):
        out = np.array([[1.0]])
        for i in range(4):
            out = np.kron(out, U2 if i == w else I2)
        return out

    def emb2(U4, w):
        return np.kron(np.eye(2 ** w), np.kron(U4, np.eye(2 ** (2 - w))))

    U = np.eye(16)
    for l in range(2):
        for b in range(3):
            U = emb1(ry(Wm[l, b, 0]), b) @ U
            U = emb1(ry(Wm[l, b, 1]), b + 1) @ U
            U = emb2(CNOT, b) @ U
    return U


def _build_bass(loop_reps=None, loop_unroll=1, empty=False):
    import concourse.bacc as bacc
    import concourse.mybir as mybir
    from concourse.tile import TileContext

    f32 = mybir.dt.float32
    bf16 = mybir.dt.bfloat16
    AF = mybir.ActivationFunctionType
    ALU = mybir.AluOpType

    nc = bacc.Bacc(None)
    blob0 = nc.dram_tensor("blob0", [128, D0], bf16, kind="ExternalInput")
    blob1 = nc.dram_tensor("blob1", [128, PCOL0 + PCOLS - D0], bf16,
                           kind="ExternalInput")
    out = nc.dram_tensor("out", [128, 256], bf16, kind="ExternalOutput")

    with TileContext(nc) as tc:
        with (
            tc.tile_pool(name="big", bufs=1) as bigpool,
            tc.tile_pool(name="work", bufs=1) as wpool,
            tc.tile_pool(name="psum", bufs=1, space="PSUM") as ppool,
        ):
            P = bigpool.tile([128, PCOL0 + PCOLS], bf16, tag="P")
            warm = bigpool.tile([128, WARM_N], bf16, tag="warm")
            wps = ppool.tile([128, WARM_N], f32, tag="wps")

            def warmup():
                # keep PE continuously busy from ~t=0 so the p-state ramp
                # (full speed after 3us) completes before the real matmuls
                # a tile must have >=1 writer to be allocated; the matmuls
                # happily consume the rest uninitialized (results discarded)
                nc.vector.memset(warm[:, 0:8], 0)
                for _ in range(WARM_MMS):
                    nc.tensor.matmul(
                        wps[:], lhsT=warm[:, 0:128], rhs=warm[:],
                        start=True, stop=True,
                    )

            def load():
                nc.sync.dma_start(out=P[:, 0:D0], in_=blob0[:, :])
                nc.sync.dma_start(out=P[:, D0:PCOL0 + PCOLS], in_=blob1[:, :])

            def body():
                qsq = wpool.tile([128, GRID], bf16, tag="qsq")
                Psq = wpool.tile([128, PCOLS], bf16, tag="Psq")
                zs = wpool.tile([128, 2 * QW], bf16, tag="zs")
                res = wpool.tile([128, 256], bf16, tag="res")
                # pad cols so the out DMA moves 512B/partition (no sub-512B
                # read-modify-write penalty); Pool is idle so memset is free
                nc.gpsimd.memset(res[:, QW:256], 0)
                # planes^2 in two pieces so chunk0's den-mms don't wait DMA1
                nc.vector.tensor_tensor(
                    Psq[:, 0:D0 - PCOL0], P[:, PCOL0:D0], P[:, PCOL0:D0],
                    ALU.mult,
                )
                nc.vector.tensor_tensor(
                    Psq[:, D0 - PCOL0:PCOLS], P[:, D0:PCOL0 + PCOLS],
                    P[:, D0:PCOL0 + PCOLS], ALU.mult,
                )

                # pass 1a (emitted first = higher scheduler priority):
                # all q matmuls + ACT squares, so the squares never queue
                # behind pass-1b/2 ACT work and the PE never stalls on them
                for ci, (r0, nr) in enumerate(CHUNKS):
                    N = nr * 32
                    c0 = r0 * 32
                    qp = ppool.tile([128, N], f32, tag=f"qp{ci}")
                    for s, (dh, dw) in enumerate(SHIFTS):
                        base = PCOL0 + (r0 + dh) * 32 + dw
                        nc.tensor.matmul(
                            qp[:], lhsT=P[:, 128 * s:128 * s + 128],
                            rhs=P[:, base:base + N],
                            start=(s == 0), stop=(s == 3),
                        )
                    nc.scalar.activation(qsq[:, c0:c0 + N], qp[:], AF.Square)

                # pass 1b: mm2, packed 4-way by column strips
                # (tile_position col-tiling): zz partition 32c+i = image i /
                # quarter c, cols 0:248 = num, cols 248:496 = 2*den
                zz = ppool.tile([128, 2 * QW], f32, tag="zz")
                for c in range(4):
                    for s, (dh, dw) in enumerate(SHIFTS):
                        base = QW * c + 32 * dh + dw
                        nc.tensor.matmul(
                            zz[32 * c:32 * c + 32, QW:2 * QW],
                            lhsT=P[:, 544:576],
                            rhs=Psq[:, base:base + QW],
                            start=(s == 0), stop=(s == 3),
                            tile_position=(0, 32 * c),
                        )
                for c in range(4):
                    nc.tensor.matmul(
                        zz[32 * c:32 * c + 32, 0:QW], lhsT=P[:, 512:544],
                        rhs=qsq[:, QW * c:QW * c + QW],
                        start=True, stop=True, tile_position=(0, 32 * c),
                    )

                # pass 2: finals.  TT-divide is not a valid DVE instruction
                # on trn2, so: reciprocal of the den half (copied out early —
                # it only depends on the den matmuls, so recip runs hidden
                # under the num matmuls), then copy(num) -> multiply.
                nc.scalar.copy(zs[:, QW:2 * QW], zz[:, QW:2 * QW])
                rden = wpool.tile([128, QW], bf16, tag="rden")
                with nc.allow_low_precision("bf16 ok at 2e-2 tolerance"):
                    nc.vector.reciprocal(rden[:], zs[:, QW:2 * QW])
                nc.scalar.copy(zs[:, 0:QW], zz[:, 0:QW])
                nc.vector.tensor_tensor(
                    res[:, 0:QW], zs[:, 0:QW], rden[:], ALU.mult,
                )
                nc.sync.dma_start(out=out[:, :], in_=res[:])

            if loop_reps is None:
                warmup()
                load()
                body()
            else:
                warmup()
                with tc.For_i(0, loop_reps, 1):
                    for _ in range(loop_unroll):
                        load()
                        body()
    nc.compile()
    return nc


def _get_bass():
    if "nc" not in _CACHE:
        _CACHE["nc"] = _build_bass()
    return _CACHE["nc"]


def _prep_inputs(img, weights_mps):
    img = np.ascontiguousarray(np.asarray(img, dtype=np.float32))
    U = _build_U(weights_mps)
    A = U[:4, :]

    Wt = np.zeros((128, PCOL0), dtype=np.float32)
    idx = np.arange(NI)
    for s, (dh, dw) in enumerate(SHIFTS):
        for pi in range(2):
            for pj in range(2):
                pl = 2 * pi + pj
                t = 4 * (2 * dh + pi) + (2 * dw + pj)
                for j in range(4):
                    Wt[4 * idx + pl, 128 * s + 4 * idx + j] = A[j, t]
    for j in range(4):
        Wt[4 * idx + j, 512 + idx] = 1.0       # num = sum_j q_j^2
    for pl in range(4):
        Wt[4 * idx + pl, 544 + idx] = 2.0      # 2*den = 2 sum planes^2

    # parity planes: [core, 128 = img*4 + (2pi+pj), 1024 = 32r + c]
    Pl = img[:, 0].reshape(BS, 32, 2, 32, 2)          # (b, r, pi, c, pj)
    Pl = Pl.transpose(0, 2, 4, 1, 3).reshape(N_CORES, 128, 1024)

    blob = np.zeros((N_CORES, 128, PCOL0 + PCOLS), dtype=np.float32)
    blob[:, :, 0:PCOL0] = Wt[None]
    blob[:, :, PCOL0:PCOL0 + 1024] = Pl
    blob16 = blob.astype(ml_dtypes.bfloat16)
    return (
        np.ascontiguousarray(blob16[:, :, 0:D0]),
        np.ascontiguousarray(blob16[:, :, D0:]),
    )


def kernel(img: np.ndarray, weights_mps: np.ndarray) -> np.ndarray:
    from concourse.bass_utils import run_bass_kernel_spmd

    blob0, blob1 = _prep_inputs(img, weights_mps)
    nc = _get_bass()
    in_maps = [{"blob0": blob0[c], "blob1": blob1[c]} for c in range(N_CORES)]
    r = run_bass_kernel_spmd(
        nc, in_maps, list(range(N_CORES)), trace=TRACE, **TRACE_KWARGS
    )
    if TRACE:
        _CACHE["last_result"] = r

    outs = np.stack([np.asarray(r.results[c]["out"]) for c in range(N_CORES)])
    # [core, 32c+i, k] -> image core*32+i, pixel-buffer col 248c+k
    res = (
        outs.astype(np.float32)[:, :, 0:QW]
        .reshape(N_CORES, 4, NI, QW)
        .transpose(0, 2, 1, 3)
        .reshape(BS, 31, 32)[:, :, :31]
    ) + np.float32(0.5)          # device ships num/(2den); host adds the 0.5
    return np.ascontiguousarray(res.reshape(BS, 1, OH * OW))


# revision 21
# speedup vs baseline: 1.9333x; 1.0684x over previous
"""Trainium2 Bass kernel for nn_Encoder_21964462752332 (parity-plane rewrite).

Math: the swap-test circuit per 4x4 patch p reduces to
    out = 0.5 + 0.5 * ||A p||^2 / ||p||^2 = (num + den) / (2 den),
with A = U[:4, :], num = ||A p||^2, den = ||p||^2 (U = 16x16 MPS orthogonal
matrix built from the 12 weights_mps floats; see _build_U).

Dataflow (per core, 32 images, SPMD over 8 cores):
  The stride-2 / kernel-4 patch extraction is re-expressed over the four
  image parity planes Pl[pi,pj][r,c] = img[2r+pi, 2c+pj] (32x32 each).
  Patch (oh,ow) tap (kh,kw) = Pl[kh%2,kw%2][oh+kh//2, ow+kw//2], so with a
  [128, 1056] planes tile (partition = image*4 + plane, col = 32*r + c,
  pixel grid padded to 32 cols incl. a garbage ow=31 so every shifted view
  is a CONTIGUOUS column range):
    q          : 4 shift-matmuls, blockdiag(A-slice) weights  -> PSUM
    q^2        : ACT Square PSUM->SBUF (bf16)
    planes^2   : DVE tensor_tensor (bf16, 2x mode)
    num+den,
    2*den      : one PSUM accumulation group = 4 shift-ones-matmuls over
                 planes^2 (M=64: rows 0:32 get 1x, rows 32:64 get 2x)
                 + 1 ones-matmul over q^2 (rows 0:32)
    out        : ACT copy PSUM->SBUF, DVE reciprocal + multiply -> bf16
  Raw pixels ship once in bf16 (~0.3 MB/core vs 2.1 MB im2col f32 before);
  all matmuls bf16 (1 cyc/row).  Output ships bf16, host upcasts.
  A few zero-weight warm-up matmuls at t=0 ramp the PE p-state early.
"""

import numpy as np
import ml_dtypes

# ---- problem geometry (hardcoded per contract) ----
BS = 256
H = W = 64
OH = OW = 31
N_CORES = 8
NI = BS // N_CORES              # 32 images per core
GRID = 32 * 31                  # padded pixel grid cols (ow=31 is garbage)
PCOL0 = 576                     # planes tile offset inside P (after weights)
PCOLS = 1056                    # 1024 real plane cols + 32 pad
SHIFTS = [(0, 0), (0, 1), (1, 0), (1, 1)]
# output-row chunks (r0, nrows): N = nrows*32 <= 512 (PSUM bank); the last
# chunk is small so the post-matmul ACT/DVE/DMA tail is short
CHUNKS = [(0, 16), (16, 15)]
QW = GRID // 4                  # mm2 column-strip (quarter) width = 248
D0 = PCOL0 + 576                # first DMA: weights + planes rows 0..18

_CACHE = {}
TRACE = False            # test.py sets this to profile
TRACE_KWARGS = {}

WARM_MMS = 13            # PE ramp warm-up matmuls on zeroed SBUF
WARM_N = 256
OUT_DMAS = [(0, 512), (512, GRID)]


def _build_U(weights_mps: np.ndarray) -> np.ndarray:
    """16x16 orthogonal MPS circuit matrix; amp index bits are MSB-first in
    local data-wire order (wire 0 = most significant)."""
    Wm = np.asarray(weights_mps, dtype=np.float64)
    I2 = np.eye(2)
    CNOT = np.array(
        [[1, 0, 0, 0], [0, 1, 0, 0], [0, 0, 0, 1], [0, 0, 1, 0]], dtype=np.float64
    )

    def ry(t):
        c, s = np.cos(t / 2.0), np.sin(t / 2.0)
        return np.array([[c, -s], [s, c]])

    def emb1(U2, w):
        out = np.array([[1.0]])
        for i in range(4):
            out = np.kron(out, U2 if i == w else I2)
        return out

    def emb2(U4, w):
        return np.kron(np.eye(2 ** w), np.kron(U4, np.eye(2 ** (2 - w))))

    U = np.eye(16)
    for l in range(2):
        for b in range(3):
            U = emb1(ry(Wm[l, b, 0]), b) @ U
            U = emb1(ry(Wm[l, b, 1]), b + 1) @ U
            U = emb2(CNOT, b) @ U
    return U


def _build_bass(loop_reps=None, loop_unroll=1, empty=False):
    import concourse.bacc as bacc
    import concourse.mybir as mybir
    from concourse.tile import TileContext

    f32 = mybir.dt.float32
    bf16 = mybir.dt.bfloat16
    AF = mybir.ActivationFunctionType
    ALU = mybir.AluOpType

    nc = bacc.Bacc(None)
    blob0 = nc.dram_tensor("blob0", [128, D0], bf16, kind="ExternalInput")
    blob1 = nc.dram_tensor("blob1", [128, PCOL0 + PCOLS - D0], bf16,
                           kind="ExternalInput")
    out = nc.dram_tensor("out", [128, 256], bf16, kind="ExternalOutput")

    with TileContext(nc) as tc:
        with (
            tc.tile_pool(name="big", bufs=1) as bigpool,
            tc.tile_pool(name="work", bufs=1) as wpool,
            tc.tile_pool(name="psum", bufs=1, space="PSUM") as ppool,
        ):
            P = bigpool.tile([128, PCOL0 + PCOLS], bf16, tag="P")
            warm = bigpool.tile([128, WARM_N], bf16, tag="warm")
            wps = ppool.tile([128, WARM_N], f32, tag="wps")

            def warmup():
                # keep PE continuously busy from ~t=0 so the p-state ramp
                # (full speed after 3us) completes before the real matmuls
                # a tile must have >=1 writer to be allocated; the matmuls
                # happily consume the rest uninitialized (results discarded)
                nc.vector.memset(warm[:, 0:8], 0)
                for _ in range(WARM_MMS):
                    nc.tensor.matmul(
                        wps[:], lhsT=warm[:, 0:128], rhs=warm[:],
                        start=True, stop=True,
                    )

            def load():
                nc.sync.dma_start(out=P[:, 0:D0], in_=blob0[:, :])
                nc.sync.dma_start(out=P[:, D0:PCOL0 + PCOLS], in_=blob1[:, :])

            def body():
                qsq = wpool.tile([128, GRID], bf16, tag="qsq")
                Psq = wpool.tile([128, PCOLS], bf16, tag="Psq")
                zs = wpool.tile([128, 2 * QW], bf16, tag="zs")
                res = wpool.tile([128, 256], bf16, tag="res")
                # pad cols so the out DMA moves 512B/partition (no sub-512B
                # read-modify-write penalty); Pool is idle so memset is free
                nc.gpsimd.memset(res[:, QW:256], 0)
                # planes^2 in two pieces so chunk0's den-mms don't wait DMA1
                nc.vector.tensor_tensor(
                    Psq[:, 0:D0 - PCOL0], P[:, PCOL0:D0], P[:, PCOL0:D0],
                    ALU.mult,
                )
                nc.vector.tensor_tensor(
                    Psq[:, D0 - PCOL0:PCOLS], P[:, D0:PCOL0 + PCOLS],
                    P[:, D0:PCOL0 + PCOLS], ALU.mult,
                )

                # pass 1a (emitted first = higher scheduler priority):
                # all q matmuls + ACT squares, so the squares never queue
                # behind pass-1b/2 ACT work and the PE never stalls on them
                for ci, (r0, nr) in enumerate(CHUNKS):
                    N = nr * 32
                    c0 = r0 * 32
                    qp = ppool.tile([128, N], f32, tag=f"qp{ci}")
                    for s, (dh, dw) in enumerate(SHIFTS):
                        base = PCOL0 + (r0 + dh) * 32 + dw
                        nc.tensor.matmul(
                            qp[:], lhsT=P[:, 128 * s:128 * s + 128],
                            rhs=P[:, base:base + N],
                            start=(s == 0), stop=(s == 3),
                        )
                    nc.scalar.activation(qsq[:, c0:c0 + N], qp[:], AF.Square)

                # pass 1b: mm2, packed 4-way by column strips
                # (tile_position col-tiling): zz partition 32c+i = image i /
                # quarter c, cols 0:248 = num, cols 248:496 = 2*den
                # separate PSUM tiles so the den copy depends only on the
                # den matmuls (whole-tile deps), not on the num matmuls
                zzd = ppool.tile([128, QW], f32, tag="zzd")
                zzn = ppool.tile([128, QW], f32, tag="zzn")
                for c in range(4):
                    for s, (dh, dw) in enumerate(SHIFTS):
                        base = QW * c + 32 * dh + dw
                        nc.tensor.matmul(
                            zzd[32 * c:32 * c + 32, :],
                            lhsT=P[:, 544:576],
                            rhs=Psq[:, base:base + QW],
                            start=(s == 0), stop=(s == 3),
                            tile_position=(0, 32 * c),
                        )
                for c in range(4):
                    nc.tensor.matmul(
                        zzn[32 * c:32 * c + 32, :], lhsT=P[:, 512:544],
                        rhs=qsq[:, QW * c:QW * c + QW],
                        start=True, stop=True, tile_position=(0, 32 * c),
                    )

                # pass 2: finals.  TT-divide is not a valid DVE instruction
                # on trn2, so: reciprocal of the den half (copied out early —
                # it only depends on the den matmuls, so recip runs hidden
                # under the num matmuls), then copy(num) -> multiply.
                nc.scalar.copy(zs[:, QW:2 * QW], zzd[:])
                rden = wpool.tile([128, QW], bf16, tag="rden")
                with nc.allow_low_precision("bf16 ok at 2e-2 tolerance"):
                    nc.vector.reciprocal(rden[:], zs[:, QW:2 * QW])
                nc.scalar.copy(zs[:, 0:QW], zzn[:])
                nc.vector.tensor_tensor(
                    res[:, 0:QW], zs[:, 0:QW], rden[:], ALU.mult,
                )
                nc.sync.dma_start(out=out[:, :], in_=res[:])

            if loop_reps is None:
                warmup()
                load()
                body()
            else:
                warmup()
                with tc.For_i(0, loop_reps, 1):
                    for _ in range(loop_unroll):
                        load()
                        body()
    nc.compile()
    return nc


def _get_bass():
    if "nc" not in _CACHE:
        _CACHE["nc"] = _build_bass()
    return _CACHE["nc"]


def _prep_inputs(img, weights_mps):
    img = np.ascontiguousarray(np.asarray(img, dtype=np.float32))
    U = _build_U(weights_mps)
    A = U[:4, :]

    Wt = np.zeros((128, PCOL0), dtype=np.float32)
    idx = np.arange(NI)
    for s, (dh, dw) in enumerate(SHIFTS):
        for pi in range(2):
            for pj in range(2):
                pl = 2 * pi + pj
                t = 4 * (2 * dh + pi) + (2 * dw + pj)
                for j in range(4):
                    Wt[4 * idx + pl, 128 * s + 4 * idx + j] = A[j, t]
    for j in range(4):
        Wt[4 * idx + j, 512 + idx] = 1.0       # num = sum_j q_j^2
    for pl in range(4):
        Wt[4 * idx + pl, 544 + idx] = 2.0      # 2*den = 2 sum planes^2

    # parity planes: [core, 128 = img*4 + (2pi+pj), 1024 = 32r + c]
    Pl = img[:, 0].reshape(BS, 32, 2, 32, 2)          # (b, r, pi, c, pj)
    Pl = Pl.transpose(0, 2, 4, 1, 3).reshape(N_CORES, 128, 1024)

    blob = np.zeros((N_CORES, 128, PCOL0 + PCOLS), dtype=np.float32)
    blob[:, :, 0:PCOL0] = Wt[None]
    blob[:, :, PCOL0:PCOL0 + 1024] = Pl
    blob16 = blob.astype(ml_dtypes.bfloat16)
    return (
        np.ascontiguousarray(blob16[:, :, 0:D0]),
        np.ascontiguousarray(blob16[:, :, D0:]),
    )


def kernel(img: np.ndarray, weights_mps: np.ndarray) -> np.ndarray:
    from concourse.bass_utils import run_bass_kernel_spmd

    blob0, blob1 = _prep_inputs(img, weights_mps)
    nc = _get_bass()
    in_maps = [{"blob0": blob0[c], "blob1": blob1[c]} for c in range(N_CORES)]
    r = run_bass_kernel_spmd(
        nc, in_maps, list(range(N_CORES)), trace=TRACE, **TRACE_KWARGS
    )
    if TRACE:
        _CACHE["last_result"] = r

    outs = np.stack([np.asarray(r.results[c]["out"]) for c in range(N_CORES)])
    # [core, 32c+i, k] -> image core*32+i, pixel-buffer col 248c+k
    res = (
        outs.astype(np.float32)[:, :, 0:QW]
        .reshape(N_CORES, 4, NI, QW)
        .transpose(0, 2, 1, 3)
        .reshape(BS, 31, 32)[:, :, :31]
    ) + np.float32(0.5)          # device ships num/(2den); host adds the 0.5
    return np.ascontiguousarray(res.reshape(BS, 1, OH * OW))
